# revision 62
# baseline (speedup 1.0000x reference)
"""Trainium2 Bass kernel v3 for nn_CrossLayerAttention_309237645906.

Reference computation (B=2, SQ=SK=2048, H=2048, NH=16, HD=128, fp32):
    q = hidden @ w_q.T + b_q                     -> [B, NH, SQ, HD]
    scores = mask + scale * q @ k                (k given as [B*NH, HD, SK])
    probs = softmax(scores)                      (fp32)
    out = (probs @ v)                            -> [B, SQ, H]
    y = out @ w_proj.T + b_proj

Sharding: 8 cores = (batch b = c//4) x (query-row subset). For the causal
mask the 512 rows of core c are the strided set {4*i + (c%4)}: every core
then has identical causal structure, so one SPMD program is work-exact:
per head, key-tile pair pa only streams query columns [64*pa, 512).

All matmuls run in bf16 (1 cycle/moving-column on PE; fp8 DoubleRow was
evaluated and rejected: p1_fp8 measures 1.9e-2 rel err vs the 2e-2 gate,
all-bf16 is ~3e-3). T-layout throughout (contraction dim on partitions,
no on-device transposes):
    qT[o, i]      = (wq stationary) @ (xT moving)       o-tile == head
    scoresT[j, i] = (k_h tile stationary) @ qT_h         per (head, j-pair)
    p = exp(scale * scoresT)   (ScalarE reads the PSUM pair directly)
    p *= mask01 diagonal strip (DVE, pairs 0-5; pairs 6-7 instead add the
                                -1e9 strip pre-exp via a tiny identity
                                matmul so the boundary-critical chain has
                                no extra DVE hop)
    S += p  (DVE bf16 adds)    per-key partial sums across j-tiles, so the
                               softmax denominator needs ONE [1,512] matmul
                               per head (ones @ S) instead of one per
                               j-tile: the moving-column cost of a matmul
                               is independent of output width, so per-pair
                               Z matmuls would cost as much as PV itself
    outT_h[d, i] += (v_h tile) @ p
    rb = bcast(1/Z) (rank-1 PE matmul, deferred one consume so the PE
                     never waits on the DVE reciprocal; z and bc tiles
                     strictly alternate through one PSUM slot)
    attnT_h = outT_h * rb      (DVE)
    y[i, o] = (attnT stationary) @ wp moving + b_proj

Schedule notes (the Tile scheduler list-schedules greedily by readiness,
tie-broken by emission order, so emission placement = priority):
  - PSUM (16KB/partition): scores pool 2x[128,2,512], PV/P3 ring
    2x[128,512] (same pool tag), one z/bc slot, one dedicated
    q-projection slot. The dedicated slot keeps the o-tile filler READY
    at head boundaries where everything else gates on the ScalarE exp
    chain; only scores tiles live in the scores ring (anything else
    there either blocks head 0 or serializes head 15).
  - o-tiles 0-3 execute during the serial-DMA prefix across the
    q/o/z slots; one o-tile rides inside each head (pair 1) as filler.
  - PV consume lag is 9 pairs (3 for the last head so its PV/normalize
    does not pile into the drain), putting Z(h) ~3 pairs into head h+2,
    long after the exp -> S-add chain drained.
  - Heads 12-15 get one early P3 accumulator's partial sums as filler
    (ps_q slot only: a second one in the ps_o ring would push op15's
    slot-wait to mult14 and serialize head 15); the remaining P3 tiles
    alternate between the retired scores pool and the PV ring, and the
    final tile splits into three independent slivers so the last Y DMA
    is short.
  - All DMA rides the SP queue jit-ordered (DMA_ENGINES and HWDGE gen
    are single serial resources; every dma_start costs ~625ns of gen, so
    small statics are packed into ONE [128,1538] bf16 tensor). wq
    eighths 4-7 queue mid-attention so their SBUF-slot waits cannot
    block the k/v stream behind them.
GPSIMD is left idle: its Q7 ISA ops and SWDGE queue crash this runtime,
and it cannot touch PSUM (tensor_tensor Add there costs 2.0ns/elem vs
DVE bf16 0.52ns/elem anyway).

Measured (TimelineSim cost model, the grading metric): 195458 ns/core,
rel err 2.98e-3; PE busy ~178us of that. Baseline v2 was 232399 ns.
The in-head o-tile filler is emitted in thirds (pairs 1/3/5) so the
greedy scheduler keeps later chunks in inventory for the head-boundary
exp-chain stalls instead of spending them on mid-head mini-stalls.
Further v3 refinements: pairs 6/7 fused into one 384-col scores plane +
ONE exp (the tail pairs' per-instruction ScalarE overhead stalled every
head boundary), and the deferred normalize flushes at pair start so its
rb/mult DVE ops never queue behind exp-gated S-adds.
"""

import sys

sys.path.insert(0, "/opt/trn_rl_repo")

import numpy as np

import concourse.bacc as bacc
import concourse.bass as bass
import concourse.mybir as mybir
import concourse.tile as tile
from concourse.bass_utils import run_bass_kernel_spmd

F32 = mybir.dt.float32
BF16 = mybir.dt.bfloat16

B, SQ, SK, H, NH = 2, 2048, 2048, 2048, 16
HD = H // NH  # 128
ROWS = 512            # query rows per core
NCORES = 8
KT = H // 128         # 16 contraction tiles for the projections
JT = SK // 128        # 16 key tiles
IT = ROWS // 128      # 4 query 128-tiles per core
SCALE = 1.0 / float(np.sqrt(HD))
MULT = mybir.AluOpType.mult
ADD = mybir.AluOpType.add
EXP = mybir.ActivationFunctionType.Exp
IDENT = mybir.ActivationFunctionType.Identity


def build_causal():
    """Per-core Bass program for the standard causal mask."""
    nc = bacc.Bacc()

    xT = nc.dram_tensor("xT", [128, KT, ROWS], BF16, kind="ExternalInput")
    wq = nc.dram_tensor("wq", [128, KT, H], BF16, kind="ExternalInput")
    bq = nc.dram_tensor("bq", [128, KT, 1], F32, kind="ExternalInput")
    key = nc.dram_tensor("key", [NH, HD, SK], BF16, kind="ExternalInput")
    value = nc.dram_tensor("value", [NH, 128, JT, HD], BF16,
                           kind="ExternalInput")
    # packed bf16 statics, one DMA instead of four (each DMA costs ~625ns
    # of serial descriptor-gen regardless of size): 64-col diagonal strips
    # of the (1/scale-premultiplied) mask [1024], identity [128], ones
    # columns [2], ones row [128]
    statics = nc.dram_tensor("statics", [128, 1538], BF16,
                             kind="ExternalInput")
    wp = nc.dram_tensor("wp", [128, KT, H], BF16, kind="ExternalInput")
    bpB = nc.dram_tensor("bpB", [128, H], F32, kind="ExternalInput")
    Y = nc.dram_tensor("Y", [ROWS, H], F32, kind="ExternalOutput")

    with tile.TileContext(nc) as tc:
        with tc.tile_pool(name="res", bufs=1) as res:
            # ---- resident tiles ----
            qT_all = res.tile([128, KT, ROWS], BF16)
            attnT_all = res.tile([128, NH, ROWS], BF16)
            statics_sb = res.tile([128, 1538], BF16)
            maskS_all = statics_sb[:, 0:1024]
            ident_sb = statics_sb[:, 1024:1152]
            ones_sb = statics_sb[:, 1152:1154]
            ones1_sb = statics_sb[0:1, 1154:1282]
            # additive -1e9 strips for j=12..15 live at [1282:1538]; the
            # 514-offset view makes maskN_all[:, 64*j:...] land there
            maskN_all = statics_sb[:, 514:1538]
            bq_all = res.tile([128, KT, 1], F32)
            bpB_all = res.tile([128, H], F32)
            xT_all = res.tile([128, KT, ROWS], BF16)

            # pools living across phases
            kv = tc.alloc_tile_pool(name="kv", bufs=5)
            pp = tc.alloc_tile_pool(name="pp", bufs=12)
            sm = tc.alloc_tile_pool(name="sm", bufs=3)
            sp = tc.alloc_tile_pool(name="sp", bufs=4)
            wqp = tc.alloc_tile_pool(name="wqp", bufs=4)
            wpp = tc.alloc_tile_pool(name="wpp", bufs=16)

            # ---- input streams on the SP DMA queue ----
            wq_sbs = []

            def fetch_wq(e):
                w_sb = wqp.tile([128, KT, 256], BF16, tag="wq", name=f"wq{e}")
                nc.sync.dma_start(w_sb, wq[:, :, 256 * e:256 * (e + 1)])
                wq_sbs.append(w_sb)
                return w_sb

            k_sbs, v_sbs = {}, {}

            def fetch_head(h):
                k_sb = kv.tile([128, SK], BF16, tag="k", name=f"k{h}")
                nc.sync.dma_start(k_sb, key[h, :, :])
                v_sb = kv.tile([128, JT, HD], BF16, tag="v", name=f"v{h}")
                nc.sync.dma_start(v_sb, value[h, :, :, :])
                k_sbs[h] = k_sb
                v_sbs[h] = v_sb

            # DMA_ENGINES is a serial FIFO: the very first chunks are tiny
            # so matmul 0 starts ~2.7us in (fixed DGE+sem overhead bound);
            # all of xT precedes k0 because o-tile 0 contracts over all of
            # it before the first scores matmul can run. k0 is split so
            # head 0's early scores pairs unblock before the whole row of
            # keys lands.
            w0 = wqp.tile([128, KT, 256], BF16, tag="wq", name="wq0")
            wq_sbs.append(w0)
            nc.sync.dma_start(w0[:, 0:1, :], wq[:, 0:1, 0:256])
            nc.sync.dma_start(xT_all[:, 0:1, :], xT[:, 0:1, :])
            nc.sync.dma_start(w0[:, 1:4, :], wq[:, 1:4, 0:256])
            nc.sync.dma_start(xT_all[:, 1:4, :], xT[:, 1:4, :])
            nc.sync.dma_start(w0[:, 4:8, :], wq[:, 4:8, 0:256])
            nc.sync.dma_start(xT_all[:, 4:8, :], xT[:, 4:8, :])
            nc.sync.dma_start(w0[:, 8:, :], wq[:, 8:, 0:256])
            nc.sync.dma_start(xT_all[:, 8:12, :], xT[:, 8:12, :])
            nc.sync.dma_start(bq_all, bq[:, :, :])
            nc.sync.dma_start(statics_sb, statics[:, :])
            nc.sync.dma_start(xT_all[:, 12:16, :], xT[:, 12:16, :])
            # k0's first half ahead of eighth 1: head 0's pairs 0-3 can
            # run while e1 (o-tiles 2/3's weights) is still in flight
            k0_sb = kv.tile([128, SK], BF16, tag="k", name="k0")
            nc.sync.dma_start(k0_sb[:, 0:1024], key[0, :, 0:1024])
            fetch_wq(1)
            nc.sync.dma_start(k0_sb[:, 1024:], key[0, :, 1024:])
            v0_sb = kv.tile([128, JT, HD], BF16, tag="v", name="v0")
            nc.sync.dma_start(v0_sb, value[0, :, :, :])
            k_sbs[0], v_sbs[0] = k0_sb, v0_sb
            fetch_wq(2)
            fetch_head(1)
            fetch_wq(3)
            fetch_head(2)
            fetch_head(3)
            fetch_head(4)
            nc.sync.dma_start(bpB_all, bpB[:, :])

            # wp streamed as (o-quarter, k-pair) tiles; half the tensor is
            # prefetched early (lands during attention), the rest streams
            # during P3 as quarters free their slots.
            wp_sbs = {}
            wp_seq = [(qt, pr) for qt in range(4) for pr in range(KT // 2)]

            def fetch_wp():
                qt, pr = wp_seq[fetch_wp.i]
                fetch_wp.i += 1
                o0 = 512 * qt
                w_sb = wpp.tile([128, 2, 512], BF16, tag="wp",
                                name=f"wp{qt}_{pr}")
                nc.sync.dma_start(
                    w_sb, wp[:, 2 * pr:2 * pr + 2, o0:o0 + 512])
                wp_sbs[(qt, pr)] = w_sb

            fetch_wp.i = 0
            for _ in range(16):
                fetch_wp()

            # ---- PSUM plan (16KB/partition): scores pool 2x[128,2,512]
            # (8KB), ps_o 2x[128,512] (PV accumulators, shared with the P3
            # accumulators via the same pool tag), ps_z 1 slot (z and bc
            # strictly alternate through it), ps_q a dedicated q-projection
            # slot so the o-tile filler is always READY work for the PE at
            # head boundaries (it must never queue behind the exp-gated
            # scores ring).
            ps_s = tc.alloc_tile_pool(name="ps_s", bufs=2, space="PSUM")
            ps_o = tc.alloc_tile_pool(name="ps_o", bufs=2, space="PSUM")
            ps_z = tc.alloc_tile_pool(name="ps_z", bufs=1, space="PSUM")
            ps_q = tc.alloc_tile_pool(name="ps_q", bufs=1, space="PSUM")

            def emit_o_tile(t, pool=None, ks=0, ke=KT):
                w_sb = wq_sbs[t // 2]
                osl = slice(128 * (t % 2), 128 * (t % 2) + 128)
                pool = pool or ps_q
                if ks == 0:
                    if pool is ps_z:
                        psq = pool.tile([128, ROWS], F32, tag="z",
                                        name=f"psq{t}")
                    elif pool is ps_o:
                        psq = pool.tile([128, ROWS], F32, tag="o",
                                        name=f"psq{t}")
                    else:
                        psq = pool.tile([128, ROWS], F32, tag="q",
                                        name=f"psq{t}")
                    emit_o_tile.psq = psq
                else:
                    psq = emit_o_tile.psq
                for k in range(ks, ke):
                    nc.tensor.matmul(psq, w_sb[:, k, osl], xT_all[:, k, :],
                                     start=(k == 0), stop=(k == KT - 1))
                if ke == KT:
                    nc.scalar.activation(qT_all[:, t, :], psq, IDENT,
                                         bias=bq_all[:, t, :])

            # software pipeline (lag of 6 j-pairs) that crosses head
            # boundaries: pv of head h's last pairs issue after head h+1's
            # first scores pairs, so the PE never waits on the exp chain.
            pend = []
            normq = []

            def flush_norm():
                while normq:
                    normq.pop(0)()

            def consume(h, pa, p2, op, S, v_sb, filler=None):
                flush_norm()
                if pa == -1:
                    # fused tail: j=12..15 live in one 384-col plane
                    for j, (c0, c1, ofs) in enumerate(
                            [(0, 128, 384), (128, 256, 384),
                             (256, 320, 448), (320, 384, 448)], start=12):
                        nc.tensor.matmul(
                            op[:, ofs:], v_sb[:, j, :], p2[:, c0:c1],
                            start=False, stop=(j == JT - 1),
                            skip_group_check=True)
                else:
                    ofs = 64 * pa
                    o_dst = op[:, ofs:]
                    for u in range(2):
                        j = 2 * pa + u
                        nc.tensor.matmul(
                            o_dst, v_sb[:, j, :], p2[:, u, :],
                            start=(j == 0), stop=(j == JT - 1),
                            skip_group_check=True)
                if pa == -1:
                    # denominator: one rank-1 matmul over the S partials,
                    # then 1/Z broadcast across partitions via a rank-1 PE
                    # matmul. The bcast is deferred to the next consume so
                    # the PE never waits on the DVE reciprocal.
                    zp = ps_z.tile([1, ROWS], F32, tag="z", name=f"z{h}")
                    nc.tensor.matmul(zp, ones_sb[:, 0:1], S,
                                     start=True, stop=True)
                    if filler is not None:
                        filler()
                    rc = sm.tile([1, ROWS], BF16, tag="rc", name=f"rc{h}")
                    with nc.allow_low_precision(reason="bf16 reciprocal"):
                        nc.vector.reciprocal(rc, zp)

                    def norm_b(h=h, rc=rc, op=op):
                        # bcast tile rides the ps_z ring (never the scores
                        # ring: its rb-copy consumer sits behind queued
                        # S-adds on the DVE, and scores allocs must not
                        # wait on that). Two PSUM operands in one
                        # tensor_tensor are rejected by the BIR verifier,
                        # so the bcast bounces through SBUF.
                        bc = ps_z.tile([128, ROWS], F32, tag="z",
                                       name=f"bc{h}")
                        nc.tensor.matmul(bc, ones1_sb, rc,
                                         start=True, stop=True)
                        rb = sm.tile([128, ROWS], BF16, tag="rb",
                                     name=f"rb{h}")
                        with nc.allow_low_precision(reason="bf16 bcast"):
                            nc.vector.tensor_copy(rb, bc)
                        with nc.allow_low_precision(reason="bf16 attn"):
                            nc.vector.tensor_tensor(attnT_all[:, h, :], op,
                                                    rb, op=MULT)

                    normq.append(norm_b)

            def emit_head(h):
                k_sb, v_sb = k_sbs.pop(h), v_sbs.pop(h)
                op = ps_o.tile([128, ROWS], F32, tag="o", name=f"o{h}")
                S = sp.tile([128, ROWS], BF16, tag="S", name=f"S{h}")

                for pa in range(JT // 2 - 2):
                    # emit any deferred normalize first: its rb/mult DVE
                    # ops must not queue behind this pair's exp-gated
                    # S-adds (DVE head-of-line would delay the op-slot
                    # free that the next head's PV needs)
                    flush_norm()
                    ofs = 64 * pa
                    W = ROWS - ofs
                    p2 = pp.tile([128, 2, W], BF16, tag="p", name=f"p{h}_{pa}")
                    # scores pair in one bank-aligned PSUM tile. The
                    # causal mask: pairs 0-5 apply it AFTER exp as a 0/1
                    # multiply on the 64-col diagonal strip (DVE, cheap,
                    # off the PE); the last two pairs keep it on the PE
                    # (identity-stationary matmul adding -1e9 pre-exp) so
                    # the boundary-critical exp -> S -> Z chain has no
                    # extra DVE hop.
                    pe_mask = pa >= 6
                    sc2 = ps_s.tile([128, 2, 512], F32, tag="s",
                                    name=f"sc{h}_{pa}")
                    for u in range(2):
                        j = 2 * pa + u
                        # strip width: even j only needs its 32 diagonal
                        # cols; odd j needs 64 (32 fully-masked + 32 diag)
                        sw = 32 if u == 0 else 64
                        nc.tensor.matmul(sc2[:, u, :W],
                                         k_sb[:, 128 * j:128 * (j + 1)],
                                         qT_all[:, h, ofs:],
                                         start=True, stop=not pe_mask,
                                         skip_group_check=True)
                        if pe_mask:
                            nc.tensor.matmul(
                                sc2[:, u, 0:sw], ident_sb,
                                maskN_all[:, 64 * j:64 * j + sw],
                                start=False, stop=True,
                                skip_group_check=True)
                    nc.scalar.activation(p2, sc2[:, :, :W], EXP, scale=SCALE)
                    if not pe_mask:
                        with nc.allow_low_precision(reason="bf16 mask"):
                            for u in range(2):
                                j = 2 * pa + u
                                sw = 32 if u == 0 else 64
                                nc.vector.tensor_tensor(
                                    p2[:, u, 0:sw], p2[:, u, 0:sw],
                                    maskS_all[:, 64 * j:64 * j + sw],
                                    op=MULT)
                    # per-key partial sums across j-tiles (bf16 DVE adds);
                    # entries sum <= 16 values in (0,1], so bf16 rounding
                    # stays ~1e-3 relative on the final denominator
                    with nc.allow_low_precision(reason="bf16 denominator"):
                        if pa == 0:
                            nc.vector.tensor_tensor(S, p2[:, 0, :],
                                                    p2[:, 1, :], op=ADD)
                        else:
                            nc.vector.tensor_tensor(S[:, ofs:], S[:, ofs:],
                                                    p2[:, 0, :], op=ADD)
                            nc.vector.tensor_tensor(S[:, ofs:], S[:, ofs:],
                                                    p2[:, 1, :], op=ADD)
                    pend.append((h, pa, p2, op, S, v_sb))
                    # ~1 head of consume lag: Z(h) lands early in head
                    # h+1, after the exp -> S-add chain for head h has
                    # drained, so the PE never waits on it. The last
                    # head shortens the lag so its PV/normalize work does
                    # not pile up into the drain where the DVE chains
                    # would serialize against an idle PE.
                    thr = {NH - 1: 3, NH - 2: 5}.get(h, 8)
                    while len(pend) > thr:
                        consume(*pend.pop(0))
                    # q-projection filler inside every head: keeps the PE
                    # fed while ScalarE works through the exp chain and
                    # produces qT for head h+4 well ahead of time (o-tiles
                    # 0..3 run during the DMA prefix). Heads 14/15 (no
                    # o-tiles left) get the first two P3 accumulators'
                    # early-k partial sums instead: attnT for k <= h-2 is
                    # final by then, the first tile rides the freed ps_q
                    # slot, and the second lands in the ps_o ring right
                    # after op15.
                    # filler emitted at pair 3: late enough that the
                    # greedy scheduler still has it in hand at the next
                    # head boundary (where the exp chain lags ~1us), and
                    # its qT activation queues mid-head instead of
                    # between the boundary exps. Only ONE early P3
                    # accumulator, on the ps_q slot: a second one in the
                    # ps_o ring would push op15's slot-wait out to mult14
                    # and serialize all of head 15.
                    # split thirds: the later chunks stay in the greedy
                    # scheduler's inventory as boundary filler instead of
                    # being consumed by mid-head mini-stalls
                    if pa == 1 and h + 4 < NH:
                        emit_o_tile(h + 4, ks=0, ke=5)
                    elif pa == 3 and h + 4 < NH:
                        emit_o_tile(h + 4, ks=5, ke=10)
                    elif pa == 5 and h + 4 < NH:
                        emit_o_tile(h + 4, ks=10, ke=KT)
                    elif h == NH - 4 and pa in (1, 3, 5):
                        if pa == 1:
                            psy = ps_q.tile([128, ROWS], F32, tag="q",
                                            name="psy0_0")
                            psy0["t0"] = psy
                        p3_partial(psy0["t0"], 0, 0,
                                   *{1: (0, 2), 3: (2, 4), 5: (4, 6)}[pa])
                    elif h == NH - 3 and pa in (1, 3, 5):
                        p3_partial(psy0["t0"], 0, 0,
                                   *{1: (6, 8), 3: (8, 9), 5: (9, 10)}[pa])
                    elif h == NH - 2 and pa in (1, 3):
                        p3_partial(psy0["t0"], 0, 0,
                                   *{1: (10, 11), 3: (11, 12)}[pa])
                    elif h == NH - 1 and pa in (1, 3):
                        p3_partial(psy0["t0"], 0, 0,
                                   *{1: (12, 13), 3: (13, 14)}[pa])
                # fused tail: all four of j=12..15 in ONE 384-col scores
                # plane and ONE exp instruction (the per-instruction
                # ScalarE overhead on the tiny tail pairs is what stalls
                # every head boundary); additive -1e9 strips pre-exp.
                p67 = pp.tile([128, 384], BF16, tag="p", name=f"p{h}_t")
                sc67 = ps_s.tile([128, 2, 512], F32, tag="s",
                                 name=f"sc{h}_t")
                for j, (c0, c1, ofs) in enumerate(
                        [(0, 128, 384), (128, 256, 384),
                         (256, 320, 448), (320, 384, 448)], start=12):
                    sw = 32 if j % 2 == 0 else 64
                    nc.tensor.matmul(sc67[:, 0, c0:c1],
                                     k_sb[:, 128 * j:128 * (j + 1)],
                                     qT_all[:, h, ofs:],
                                     start=True, stop=False,
                                     skip_group_check=True)
                    nc.tensor.matmul(
                        sc67[:, 0, c0:c0 + sw], ident_sb,
                        maskN_all[:, 64 * j:64 * j + sw],
                        start=False, stop=True,
                        skip_group_check=True)
                nc.scalar.activation(p67, sc67[:, 0, 0:384], EXP,
                                     scale=SCALE)
                with nc.allow_low_precision(reason="bf16 denominator"):
                    nc.vector.tensor_tensor(S[:, 384:], S[:, 384:],
                                            p67[:, 0:128], op=ADD)
                    nc.vector.tensor_tensor(S[:, 384:], S[:, 384:],
                                            p67[:, 128:256], op=ADD)
                    nc.vector.tensor_tensor(S[:, 448:], S[:, 448:],
                                            p67[:, 256:320], op=ADD)
                    nc.vector.tensor_tensor(S[:, 448:], S[:, 448:],
                                            p67[:, 320:384], op=ADD)
                pend.append((h, -1, p67, op, S, v_sb))
                thr = {NH - 1: 3, NH - 2: 5}.get(h, 8)
                while len(pend) > thr:
                    consume(*pend.pop(0))
                if h + 5 < NH:
                    fetch_head(h + 5)
                if h in (1, 3, 5, 7):
                    # wq eighths 4-7 queue mid-attention: their wqp slots
                    # free as o-tiles retire, so queueing them earlier
                    # would block the serial DMA FIFO (and the k/v stream)
                    fetch_wq((h + 7) // 2)

            # ---- phase 3 helpers (interleaved with the attention tail) ----
            ypo = tc.alloc_tile_pool(name="ypo", bufs=3)
            psy0 = {}

            def p3_evict(psy, qt, it, hs, he, eng=None):
                # psy is the accumulation REGION (width he-hs, offset 0)
                o0 = 512 * qt
                y_sb = ypo.tile([128, he - hs], F32, tag="ysb",
                                name=f"y{qt}_{it}_{hs}")
                nc.vector.scalar_tensor_tensor(
                    y_sb, psy[:, 0:he - hs], 1.0,
                    bpB_all[:, o0 + hs:o0 + he], MULT, ADD)
                (eng or nc.sync).dma_start(
                    Y[128 * it:128 * (it + 1), o0 + hs:o0 + he], y_sb)

            def p3_partial(psy, qt, it, ks, ke):
                for k in range(ks, ke):
                    w_sb = wp_sbs[(qt, k // 2)]
                    nc.tensor.matmul(
                        psy, attnT_all[:, k, 128 * it:128 * (it + 1)],
                        w_sb[:, k % 2, :],
                        start=(k == 0), stop=False)

            # interleave: o-tiles 0-3 run during the DMA prefix (spread
            # across the free ps_q/ps_o/ps_s banks so they can overlap),
            # then one o-tile rides inside each head
            emit_o_tile(0)
            emit_o_tile(1, ps_o)
            emit_o_tile(2, ps_z)
            emit_o_tile(3)
            for h in range(NH):
                emit_head(h)
            # drain the pipeline; the last consume (h15 normalize) gets the
            # first P3 accumulators' mid-k matmuls as PE filler between
            # its Z matmul and the reciprocal-gated bcast

            def p3_first():
                p3_partial(psy0["t0"], 0, 0, 14, 15)

            while pend:
                ent = pend.pop(0)
                if ent[1] == -1 and ent[0] == NH - 1:
                    consume(*ent, filler=p3_first)
                else:
                    consume(*ent)
            flush_norm()

            # ---- phase 3: output projection ----
            # it-tile outer so each [128, 512] result evicts (and its Y DMA
            # streams out) while the next accumulates; the accumulators
            # share the PV PSUM ring (tag "o").
            w_sb = wp_sbs[(0, 7)]
            psy = psy0["t0"]
            nc.tensor.matmul(psy, attnT_all[:, 15, 0:128],
                             w_sb[:, 1, :], start=False, stop=True)
            p3_evict(psy, 0, 0, 0, 512)
            n_p3 = 0

            def p3_psy(name, width=ROWS):
                nonlocal n_p3
                if n_p3 % 2 == 0:
                    psy = ps_s.tile([128, 2, 512], F32, tag="s",
                                    name=name)[:, 0, :]
                else:
                    psy = ps_o.tile([128, ROWS], F32, tag="o", name=name)
                n_p3 += 1
                return psy[:, 0:width]

            for qt in range(4):
                for it in range(IT):
                    if qt == 0 and it == 0:
                        continue
                    last = (qt == 3 and it == IT - 1)
                    # the final tile splits into three INDEPENDENT
                    # accumulators so each sliver's eviction DMA overlaps
                    # the next sliver's accumulation and the very last Y
                    # DMA is a short one
                    splits = ([(0, 256), (256, 448), (448, 512)] if last
                              else [(0, 512)])
                    for hs, he in splits:
                        psy = p3_psy(f"psy{qt}_{it}_{hs}", he - hs)
                        for k in range(KT):
                            w_sb = wp_sbs[(qt, k // 2)]
                            att = attnT_all[:, k, 128 * it:128 * (it + 1)]
                            nc.tensor.matmul(
                                psy, att, w_sb[:, k % 2, hs:he],
                                start=(k == 0), stop=(k == KT - 1))
                        p3_evict(psy, qt, it, hs, he)
                if it == IT - 1:
                    # stream quarter qt+2 now that qt's slots are free
                    for _ in range(KT // 2):
                        if fetch_wp.i < len(wp_seq):
                            fetch_wp()
            ps_q.release()
            ps_z.release()
            ps_o.release()
            ps_s.release()
            ypo.release()
            wpp.release()
            wqp.release()
            sp.release()
            sm.release()
            pp.release()
            kv.release()

    nc.compile()
    return nc


def build_general():
    """Fallback for a non-causal mask: full maskT, per-pair Z matmuls."""
    nc = bacc.Bacc()

    xT = nc.dram_tensor("xT", [128, KT, ROWS], BF16, kind="ExternalInput")
    wq = nc.dram_tensor("wq", [128, KT, H], BF16, kind="ExternalInput")
    bq = nc.dram_tensor("bq", [128, KT, 1], F32, kind="ExternalInput")
    key = nc.dram_tensor("key", [NH, HD, SK], BF16, kind="ExternalInput")
    value = nc.dram_tensor("value", [NH, 128, JT, HD], BF16,
                           kind="ExternalInput")
    maskT = nc.dram_tensor("maskT", [128, JT, ROWS], F32,
                           kind="ExternalInput")
    wp = nc.dram_tensor("wp", [128, KT, H], BF16, kind="ExternalInput")
    bpB = nc.dram_tensor("bpB", [128, H], F32, kind="ExternalInput")
    onesd = nc.dram_tensor("onesd", [128, 2, 1], BF16, kind="ExternalInput")
    ones1d = nc.dram_tensor("ones1d", [1, 128], BF16, kind="ExternalInput")
    Y = nc.dram_tensor("Y", [ROWS, H], F32, kind="ExternalOutput")

    with tile.TileContext(nc) as tc:
        with tc.tile_pool(name="res", bufs=1) as res:
            qT_all = res.tile([128, KT, ROWS], BF16)
            attnT_all = res.tile([128, NH, ROWS], BF16)
            maskT_all = res.tile([128, JT, ROWS], F32)
            bq_all = res.tile([128, KT, 1], F32)
            bpB_all = res.tile([128, H], F32)
            ones_sb = res.tile([128, 2, 1], BF16)
            ones1_sb = res.tile([1, 128], BF16)
            xT_all = res.tile([128, KT, ROWS], BF16)

            kv = tc.alloc_tile_pool(name="kv", bufs=5)
            tp = tc.alloc_tile_pool(name="tp", bufs=3)
            pp = tc.alloc_tile_pool(name="pp", bufs=7)
            sm = tc.alloc_tile_pool(name="sm", bufs=2)
            wqp = tc.alloc_tile_pool(name="wqp", bufs=4)
            # fp32 maskT costs 32KB/partition, so only one wp quarter is
            # resident at a time here
            wpp = tc.alloc_tile_pool(name="wpp", bufs=8)

            wq_sbs = []

            def fetch_wq(e):
                w_sb = wqp.tile([128, KT, 256], BF16, tag="wq", name=f"wq{e}")
                nc.sync.dma_start(w_sb, wq[:, :, 256 * e:256 * (e + 1)])
                wq_sbs.append(w_sb)

            k_sbs, v_sbs = {}, {}

            def fetch_head(h):
                k_sb = kv.tile([128, SK], BF16, tag="k", name=f"k{h}")
                nc.sync.dma_start(k_sb, key[h, :, :])
                v_sb = kv.tile([128, JT, HD], BF16, tag="v", name=f"v{h}")
                nc.sync.dma_start(v_sb, value[h, :, :, :])
                k_sbs[h] = k_sb
                v_sbs[h] = v_sb

            w0 = wqp.tile([128, KT, 256], BF16, tag="wq", name="wq0")
            wq_sbs.append(w0)
            nc.sync.dma_start(w0[:, 0:4, :], wq[:, 0:4, 0:256])
            nc.sync.dma_start(xT_all[:, 0:2, :], xT[:, 0:2, :])
            nc.sync.dma_start(w0[:, 4:8, :], wq[:, 4:8, 0:256])
            nc.sync.dma_start(xT_all[:, 2:4, :], xT[:, 2:4, :])
            nc.sync.dma_start(xT_all[:, 4:8, :], xT[:, 4:8, :])
            nc.sync.dma_start(w0[:, 8:, :], wq[:, 8:, 0:256])
            nc.sync.dma_start(xT_all[:, 8:12, :], xT[:, 8:12, :])
            nc.sync.dma_start(xT_all[:, 12:16, :], xT[:, 12:16, :])
            nc.sync.dma_start(bq_all, bq[:, :, :])
            nc.sync.dma_start(ones_sb, onesd[:, :, :])
            nc.sync.dma_start(ones1_sb, ones1d[:, :])
            fetch_head(0)
            fetch_wq(1)
            fetch_head(1)
            fetch_wq(2)
            fetch_wq(3)
            fetch_head(2)
            fetch_wq(4)
            fetch_head(3)
            fetch_wq(5)
            nc.sync.dma_start(maskT_all[:, 0:8, :], maskT[:, 0:8, :])
            fetch_wq(6)
            fetch_head(4)
            fetch_wq(7)
            nc.sync.dma_start(maskT_all[:, 8:16, :], maskT[:, 8:16, :])
            nc.sync.dma_start(bpB_all, bpB[:, :])

            wp_sbs = {}
            wp_seq = [(qt, pr) for qt in range(4) for pr in range(KT // 2)]

            def fetch_wp():
                qt, pr = wp_seq[fetch_wp.i]
                fetch_wp.i += 1
                o0 = 512 * qt
                w_sb = wpp.tile([128, 2, 512], BF16, tag="wp",
                                name=f"wp{qt}_{pr}")
                nc.sync.dma_start(
                    w_sb, wp[:, 2 * pr:2 * pr + 2, o0:o0 + 512])
                wp_sbs[(qt, pr)] = w_sb

            fetch_wp.i = 0
            for _ in range(8):
                fetch_wp()

            ps_s = tc.alloc_tile_pool(name="ps_s", bufs=4, space="PSUM")
            ps_o = tc.alloc_tile_pool(name="ps_o", bufs=2, space="PSUM")
            ps_z = tc.alloc_tile_pool(name="ps_z", bufs=2, space="PSUM")

            def emit_o_tile(t):
                w_sb = wq_sbs[t // 2]
                osl = slice(128 * (t % 2), 128 * (t % 2) + 128)
                psq = ps_s.tile([128, ROWS], F32, tag="s", name=f"psq{t}")
                for k in range(KT):
                    nc.tensor.matmul(psq, w_sb[:, k, osl], xT_all[:, k, :],
                                     start=(k == 0), stop=(k == KT - 1))
                nc.scalar.activation(qT_all[:, t, :], psq, IDENT,
                                     bias=bq_all[:, t, :])

            pend = []

            def consume(h, pa, p2, op, zp, v_sb):
                for u in range(2):
                    j = 2 * pa + u
                    nc.tensor.matmul(
                        op, v_sb[:, j, :], p2[:, u, :],
                        start=(j == 0), stop=(j == JT - 1),
                        skip_group_check=True)
                    nc.tensor.matmul(
                        zp, ones_sb[:, 0, :], p2[:, u, :],
                        start=(j == 0), stop=(j == JT - 1),
                        skip_group_check=True)
                if pa == JT // 2 - 1:
                    rc = sm.tile([1, ROWS], BF16, tag="rc", name=f"rc{h}")
                    with nc.allow_low_precision(reason="bf16 reciprocal"):
                        nc.vector.reciprocal(rc, zp)
                    bc = ps_s.tile([128, ROWS], F32, tag="s", name=f"bc{h}")
                    nc.tensor.matmul(bc, ones1_sb, rc, start=True, stop=True)
                    rb = sm.tile([128, ROWS], BF16, tag="rb", name=f"rb{h}")
                    with nc.allow_low_precision(reason="bf16 bcast"):
                        nc.vector.tensor_copy(rb, bc)
                    with nc.allow_low_precision(reason="bf16 attn"):
                        nc.vector.tensor_tensor(attnT_all[:, h, :], op, rb,
                                                op=MULT)

            def emit_head(h):
                k_sb, v_sb = k_sbs.pop(h), v_sbs.pop(h)
                zp = ps_z.tile([1, ROWS], F32, tag="z", name=f"z{h}")
                op = ps_o.tile([128, ROWS], F32, tag="o", name=f"o{h}")

                for pa in range(JT // 2):
                    p2 = pp.tile([128, 2, ROWS], BF16, tag="p",
                                 name=f"p{h}_{pa}")
                    t2 = tp.tile([128, 2, ROWS], BF16, tag="t",
                                 name=f"t{h}_{pa}")
                    for u in range(2):
                        j = 2 * pa + u
                        sc = ps_s.tile([128, ROWS], F32, tag="s",
                                       name=f"sc{h}_{j}")
                        nc.tensor.matmul(sc,
                                         k_sb[:, 128 * j:128 * (j + 1)],
                                         qT_all[:, h, :],
                                         start=True, stop=True)
                        nc.vector.scalar_tensor_tensor(
                            t2[:, u, :], sc, 1.0,
                            maskT_all[:, j, :], MULT, ADD)
                    nc.scalar.activation(p2, t2, EXP, scale=SCALE)
                    pend.append((h, pa, p2, op, zp, v_sb))
                    if len(pend) > 5:
                        consume(*pend.pop(0))
                if h + 5 < NH:
                    fetch_head(h + 5)

            done_h = 0
            for t in range(KT):
                emit_o_tile(t)
                if t % 2 == 1 and done_h < 7:
                    emit_head(done_h)
                    done_h += 1
            for h in range(done_h, NH):
                emit_head(h)
            while pend:
                consume(*pend.pop(0))

            ps_z.release()

            ps_y = tc.alloc_tile_pool(name="ps_y", bufs=2, space="PSUM")
            with tc.tile_pool(name="ypo", bufs=3) as ypo:
                for qt in range(4):
                    o0 = 512 * qt
                    for it in range(IT):
                        last = (qt == 3 and it == IT - 1)
                        psy = ps_y.tile([128, 512], F32, tag="y",
                                        name=f"psy{qt}_{it}")
                        for hs, he in ([(0, 256), (256, 512)] if last
                                       else [(0, 512)]):
                            for pr in range(KT // 2):
                                w_sb = wp_sbs[(qt, pr)]
                                for kk in range(2):
                                    k = 2 * pr + kk
                                    att = attnT_all[:, k,
                                                    128 * it:128 * (it + 1)]
                                    nc.tensor.matmul(
                                        psy[:, hs:he], att,
                                        w_sb[:, kk, hs:he],
                                        start=(k == 0), stop=(k == KT - 1))
                            y_sb = ypo.tile([128, he - hs], F32, tag="ysb",
                                            name=f"y{qt}_{it}_{hs}")
                            nc.vector.scalar_tensor_tensor(
                                y_sb, psy[:, hs:he], 1.0,
                                bpB_all[:, o0 + hs:o0 + he], MULT, ADD)
                            nc.sync.dma_start(
                                Y[128 * it:128 * (it + 1),
                                  o0 + hs:o0 + he], y_sb)
                    for _ in range(KT // 2):
                        if fetch_wp.i < len(wp_seq):
                            fetch_wp()
            ps_y.release()
            ps_o.release()
            ps_s.release()
            wpp.release()
            wqp.release()
            sm.release()
            pp.release()
            tp.release()
            kv.release()

    nc.compile()
    return nc


_CACHE = {}


def _get_nc(causal):
    if causal not in _CACHE:
        _CACHE[causal] = build_causal() if causal else build_general()
    return _CACHE[causal]


def _is_causal(attention_mask):
    """True if the mask is exactly the standard causal additive mask."""
    m = attention_mask
    if m.shape != (B, 1, SQ, SK):
        return False
    m0 = np.asarray(m[0, 0])
    tri = np.tril(np.ones((SQ, SK), dtype=bool))
    ref = np.where(tri, np.float32(0.0), np.float32(-1e9))
    if not np.array_equal(m0, ref):
        return False
    for b in range(1, B):
        if not np.array_equal(np.asarray(m[b, 0]), m0):
            return False
    return True


def _arr(x, np_dt):
    return np.ascontiguousarray(x.astype(np_dt))


def kernel(hidden_states, key, value, attention_mask, w_q, b_q, w_proj,
           b_proj, _trace=False):
    import ml_dtypes
    BF = ml_dtypes.bfloat16

    hidden_states = np.asarray(hidden_states)
    key = np.asarray(key)
    value = np.asarray(value)
    attention_mask = np.asarray(attention_mask)
    w_q = np.asarray(w_q, dtype=np.float32)
    b_q = np.asarray(b_q, dtype=np.float32)
    w_proj = np.asarray(w_proj, dtype=np.float32)
    b_proj = np.asarray(b_proj, dtype=np.float32)

    causal = _is_causal(attention_mask)
    nc = _get_nc(causal)

    # weights arranged [p, a, o] = w.T[a*128+p, o]
    wq_arr = _arr(w_q.T.reshape(KT, 128, H).transpose(1, 0, 2), BF)
    wp_arr = _arr(w_proj.T.reshape(KT, 128, H).transpose(1, 0, 2), BF)
    bq_arr = _arr(b_q.reshape(KT, 128).T[:, :, None], np.float32)
    bp_arr = _arr(np.broadcast_to(b_proj[None, :], (128, H)), np.float32)
    inv_scale = np.float32(1.0 / SCALE)

    key_b = [_arr(key[b * NH:(b + 1) * NH], BF) for b in range(B)]
    # value arranged [h, p, j, d] = value[b, h, j*128+p, d]
    val_b = [
        _arr(value[b].reshape(NH, JT, 128, HD).transpose(0, 2, 1, 3), BF)
        for b in range(B)
    ]

    def core_rows(c):
        b, cc = c // 4, c % 4
        if causal:
            return b, cc + 4 * np.arange(ROWS)
        return b, np.arange(ROWS * cc, ROWS * cc + ROWS)

    in_maps = []
    for c in range(NCORES):
        b, rows = core_rows(c)
        xT_c = hidden_states[b][rows, :].T  # [H, ROWS]
        xT_arr = _arr(xT_c.reshape(KT, 128, ROWS).transpose(1, 0, 2), BF)
        m_c = (attention_mask[b, 0][rows, :].T * inv_scale)  # [SK, ROWS]
        im = dict(
            xT=xT_arr, wq=wq_arr, bq=bq_arr, key=key_b[b], value=val_b[b],
            wp=wp_arr, bpB=bp_arr,
        )
        if causal:
            # packed statics: 64-col 0/1 mask strips
            # (mask01[p, 64j+c] = visible(128j+p, 64(j//2)+c)), identity,
            # ones columns, ones row
            st = np.empty((128, 1538), dtype=np.float32)
            for j in range(JT):
                c0 = 64 * (j // 2)
                st[:, 64 * j:64 * (j + 1)] = (
                    m_c[128 * j:128 * (j + 1), c0:c0 + 64] == 0.0)
            st[:, 1024:1152] = np.eye(128, dtype=np.float32)
            st[:, 1152:1282] = 1.0
            for j in range(12, JT):
                c0 = 64 * (j // 2)
                st[:, 1282 + 64 * (j - 12):1282 + 64 * (j - 11)] = \
                    m_c[128 * j:128 * (j + 1), c0:c0 + 64]
            im["statics"] = _arr(st, BF)
        else:
            im["onesd"] = np.ones((128, 2, 1), dtype=BF)
            im["ones1d"] = np.ones((1, 128), dtype=BF)
            im["maskT"] = _arr(
                m_c.reshape(JT, 128, ROWS).transpose(1, 0, 2), np.float32)
        in_maps.append(im)

    kw = {}
    if _trace:
        kw = dict(trace=True, trace_cores=list(range(NCORES)),
                  stitch_traces=False)
    res = run_bass_kernel_spmd(nc, in_maps, core_ids=list(range(NCORES)), **kw)
    if _trace:
        kernel._last_result = res

    out = np.empty((B, SQ, H), dtype=np.float32)
    for c in range(NCORES):
        b, rows = core_rows(c)
        out[b][rows, :] = res.results[c]["Y"]
    return out


if __name__ == "__main__":
    pass
